# revision 33
# baseline (speedup 1.0000x reference)
"""Trainium2 Bass kernel for nn_Correlation (FlowNet-style cost volume).

Problem: input1/input2 [8, 256, 96, 128] f32 ->
         out [8, 441, 96, 128] f32
  out[b, 21*i+j, h, w] = leaky_relu_0.1( (1/256) * sum_c
        in1[b,c,h,w] * in2pad[b,c, h+2i, w+2j] )       (pad 20 each side)

Strategy (data-parallel over B across 8 cores; per core = 1 sample):
  * in2 kept in SBUF as a zero-padded full-res image [136, 168] (bf16,
    per 128-channel chunk). Parity selection happens in the matmul APs
    (stride-2 free dims) -- no on-chip rearrangement at all.
  * Pixel tile = 128 pixels (8 parity rows x 16 parity cols) per
    (block-row k, h-parity hp, w-parity wp, w-block wb). PE computes
    band[pixel, (r, v)] over the 28x36 dilated window (contract C=256,
    bf16, fp32 PSUM, two 504-col PSUM banks).
  * leaky_relu = max(0.1x, x) fused into the PSUM->SBUF pass (DVE stt /
    scalar ACT Lrelu); 1/256 pre-folded into in1 (power of two, exact).
  * The per-pixel alignment base (he*36+we) is baked into the DRAM
    scratch *write* AP (per-partition stagger is legal on the flat DRAM
    side), so the gather back is a plain 2-dim AP with 1.5KB contiguous
    runs -- 1 gather per (k,hp,wp,wb) instead of per-(he,wb).
  * Gathered rows hold (i, v=36) windows; PE transpose selects (i, j<21)
    via a 2-free-dim stationary AP and emits d-major chunks of 126.
  * Output bf16, converted to f32 on host.
"""

import numpy as np

import concourse.bass as bass
import concourse.mybir as mybir
from concourse.tile import TileContext
from concourse.bass_utils import run_bass_kernel_spmd
from concourse.masks import make_identity

DT = mybir.dt
AF = mybir.ActivationFunctionType
ALU = mybir.AluOpType

# ---- problem geometry ----
B, C, H, W = 8, 256, 96, 128
NP = 21                      # displacements per axis
ND = NP * NP                 # 441
CC = 2                       # C chunks of 128
HW = H * W

IMG_H, IMG_W = H + 40, W + 40      # 136 x 168 padded full-res image
IMG_F = IMG_H * IMG_W              # 22848
NBK = 6                            # block rows of 16 full-res rows
NR, NV = 28, 36                    # window rows / cols (parity space)
BAND = NR * NV                     # 1008
RUN = (NP - 1) * NV + NP           # 741 contiguous gather run (t = i*36+j)
SPITCH = 1032                      # scratch row pitch (>= 1029: no row overlap)
SBASE = 7 * NV + 15                # 267 stagger headroom
SROWS = 129                        # scratch rows (>= (SBASE+127*SPITCH+BAND)/SPITCH)
STG1_F = CC * 16 * W               # 4096
# transpose d-chunks (i0, ni): nd = 21*ni
CHUNKS = [(0, 6), (6, 6), (12, 6), (18, 3)]

_MAX_WAITS = 1


def _split_excess_waits(nc):
    """This walrus build accepts only ONE sync-wait per instruction; Tile
    emits multi-waits. Hoist excess waits onto same-engine NOPs inserted
    right before the over-subscribed instruction."""
    nid = 0
    for f in nc.m.functions:
        for blk in f.blocks:
            insts = list(blk.instructions)
            out = []
            changed = False
            for inst in insts:
                si = inst.sync_info
                if si is not None and si.on_wait and len(si.on_wait) > _MAX_WAITS:
                    waits = list(si.on_wait)
                    extra, keep = waits[:-_MAX_WAITS], waits[-_MAX_WAITS:]
                    for k in range(0, len(extra), _MAX_WAITS):
                        nop = mybir.InstNoOp(name=f"I-waitsplit-{nid}", ins=[], outs=[])
                        nid += 1
                        nop.engine = inst.engine
                        nop.sync_info = mybir.SyncInfo(
                            on_wait=extra[k : k + _MAX_WAITS], on_update=[]
                        )
                        out.append(nop)
                        changed = True
                    si.on_wait = keep
                    inst.sync_info = si
                out.append(inst)
            if changed:
                blk.instructions = out
    return nc


def _ap(t, off_extra, dims):
    return bass.AP(tensor=t.tensor, offset=t.offset + off_extra, ap=dims)


def _build_nc(waitsplit=True, use_lrelu=False):
    nc = bass.Bass()
    in1_d = nc.dram_tensor("in1", [C, H, W], DT.float32, kind="ExternalInput")
    in2_d = nc.dram_tensor("in2", [C, H, W], DT.float32, kind="ExternalInput")
    out_d = nc.dram_tensor("out", [ND, H, W], DT.bfloat16, kind="ExternalOutput")

    with TileContext(nc) as tc:
        with (
            tc.tile_pool(name="constp", bufs=1) as constp,
            tc.tile_pool(name="stg1p", bufs=2) as stg1p,
            tc.tile_pool(name="bandp", bufs=4) as bandp,
            tc.tile_pool(name="aligp", bufs=3) as aligp,
            tc.tile_pool(name="outp", bufs=2) as outp,
            tc.tile_pool(name="psp", bufs=3, space="PSUM") as psp,
            tc.tile_pool(name="trpp", bufs=2, space="PSUM") as trpp,
            tc.tile_pool(name="dramp", bufs=10, space="DRAM") as dramp,
        ):
            ident = constp.tile([128, 128], DT.bfloat16)
            make_identity(nc, ident)

            # persistent padded in2 images (one per 128-channel chunk)
            img = [constp.tile([128, IMG_F], DT.bfloat16, name=f"img{cc}") for cc in range(CC)]
            def memset_borders():
                # zero borders: top/bottom row bands, left/right col bands
                # (Pool engine: keeps DVE free for the prologue blk copies)
                for cc in range(CC):
                    t = img[cc]
                    nc.gpsimd.memset(_ap(t, 0, [[IMG_F, 128], [1, 20 * IMG_W]]), 0.0)
                    nc.gpsimd.memset(
                        _ap(t, (IMG_H - 20) * IMG_W, [[IMG_F, 128], [1, 20 * IMG_W]]),
                        0.0,
                    )
                    nc.gpsimd.memset(
                        _ap(t, 20 * IMG_W, [[IMG_F, 128], [IMG_W, 96], [1, 20]]), 0.0
                    )
                    nc.gpsimd.memset(
                        _ap(
                            t,
                            20 * IMG_W + 20 + W,
                            [[IMG_F, 128], [IMG_W, 96], [1, 20]],
                        ),
                        0.0,
                    )

            def load_img_group(g):
                """in2 full-res rows [16g, 16g+16) -> img rows [16g+20, ...)."""
                for cc in range(CC):
                    nc.gpsimd.dma_start(
                        _ap(
                            img[cc],
                            (16 * g + 20) * IMG_W + 20,
                            [[IMG_F, 128], [IMG_W, 16], [1, W]],
                        ),
                        in2_d[cc * 128 : (cc + 1) * 128, 16 * g : 16 * g + 16, :],
                    )

            def load_stg1(k):
                """in1 rows [16k, 16k+16) raw row-major staging."""
                t = stg1p.tile([128, STG1_F], DT.bfloat16, name="stg1")
                for cc in range(CC):
                    nc.gpsimd.dma_start(
                        _ap(t, cc * 16 * W, [[STG1_F, 128], [1, 16 * W]]),
                        in1_d[cc * 128 : (cc + 1) * 128, 16 * k : 16 * k + 16, :],
                    )
                return t

            def build_blk(t):
                """Block stg1 into [c, (cc,hp,wp,wb)*128 pixels] weight tiles,
                prescaled by 1/C (folded into the copy)."""
                blk = stg1p.tile([128, STG1_F], DT.bfloat16, name="in1blk")
                n = 0
                for cc in range(CC):
                    for hp in range(2):
                        for wp in range(2):
                            for wb in range(4):
                                src = _ap(
                                    t,
                                    cc * 16 * W + hp * W + 32 * wb + wp,
                                    [[STG1_F, 128], [2 * W, 8], [2, 16]],
                                )
                                dst = _ap(
                                    blk,
                                    (((cc * 2 + hp) * 2 + wp) * 4 + wb) * 128,
                                    [[STG1_F, 128], [1, 128]],
                                )
                                if n % 2 == 0:
                                    nc.vector.tensor_scalar_mul(dst, src, 1.0 / C)
                                else:
                                    nc.scalar.activation(
                                        dst, src, AF.Copy, scale=1.0 / C
                                    )
                                n += 1
                return blk

            # prologue: in1 slab first (its blk copies gate the first matmuls),
            # then img groups; border memsets ride the Pool queue after the
            # load dispatches
            raw1 = {0: load_stg1(0)}
            for g in range(4):
                load_img_group(g)
            memset_borders()
            stg1 = {0: build_blk(raw1[0])}

            NS = NBK * 4  # 24 supertiles, s = k*4 + hp*2 + wp
            state = {}  # s -> dict with scratch tiles / alig / dense / meta
            out_t = {}

            def stage_a_wb(s, wb):
                """4 matmuls + band copy + scratch write for one wb tile."""
                k, hp, wp = s // 4, (s // 2) % 2, s % 2
                st = state.setdefault(s, {"k": k, "hp": hp, "wp": wp, "scr": [], "dn": []})
                ps = psp.tile([128, 1024], DT.float32, name="ps")
                for bank in range(2):
                    for cc in range(CC):
                        lhsT = _ap(
                            stg1[k],
                            (((cc * 2 + hp) * 2 + wp) * 4 + wb) * 128,
                            [[STG1_F, 128], [1, 128]],
                        )
                        rhs = _ap(
                            img[cc],
                            (16 * k + hp + 2 * 14 * bank) * IMG_W + 32 * wb + wp,
                            [[IMG_F, 128], [2 * IMG_W, 14], [2, NV]],
                        )
                        nc.tensor.matmul(
                            _ap(ps, 512 * bank, [[1024, 128], [1, 504]]),
                            lhsT,
                            rhs,
                            start=(cc == 0),
                            stop=(cc == CC - 1),
                        )
                band = bandp.tile([128, BAND], DT.bfloat16, name="band")
                src = _ap(ps, 0, [[1024, 128], [512, 2], [1, 504]])
                dst = _ap(band, 0, [[BAND, 128], [504, 2], [1, 504]])
                if wb % 2 == 0:
                    nc.vector.tensor_copy(dst, src)
                else:
                    nc.scalar.activation(dst, src, AF.Copy)
                scr = dramp.tile([SROWS, SPITCH], DT.bfloat16, name="scr")
                wdst = _ap(
                    scr,
                    SBASE,
                    [[16 * SPITCH - NV, 8], [SPITCH - 1, 16], [1, BAND]],
                )
                # scratch writes ride the gpsimd ring (bf16->bf16, no cast
                # needed) so the latency-critical gathers own sync/scalar
                nc.gpsimd.dma_start(wdst, band[:, :])
                st["scr"].append(scr)

            def stage_gather(s):
                st = state[s]
                alig = aligp.tile([128, 4 * RUN], DT.bfloat16, name="alig")
                for wb in range(4):
                    gsrc = _ap(st["scr"][wb], SBASE, [[SPITCH, 128], [1, RUN]])
                    gdst = _ap(alig, wb * RUN, [[4 * RUN, 128], [1, RUN]])
                    eng = nc.scalar if wb % 2 else nc.sync
                    eng.dma_start(gdst, gsrc)
                st["alig"] = alig

            def stage_compact(s):
                """(i, v<36) -> dense (i, j<21) with fused leaky_relu; the
                transpose stationary APs must be single-free-dim (walrus)."""
                st = state[s]
                alig = st["alig"]
                for wb in range(4):
                    dn = aligp.tile([128, ND], DT.bfloat16, name="dense", bufs=13)
                    csrc = _ap(alig, wb * RUN, [[4 * RUN, 128], [NV, NP], [1, NP]])
                    cdst = _ap(dn, 0, [[ND, 128], [1, ND]])
                    if use_lrelu and wb % 2:
                        nc.scalar.activation(cdst, csrc, AF.Lrelu, alpha=0.1)
                    else:
                        nc.vector.scalar_tensor_tensor(
                            cdst, csrc, 0.1, csrc, ALU.mult, ALU.max
                        )
                    st["dn"].append(dn)

            def stage_tp_wb(s, wb):
                """4 transposes + 4 parity-scatter copies for one wb tile."""
                st = state[s]
                k, hp, wp = st["k"], st["hp"], st["wp"]
                if s % 4 == 0 and wb == 0:
                    out_t[k] = outp.tile([128, 4 * 16 * W], DT.bfloat16, name="outt")
                ot = out_t[k]
                dn = st["dn"][wb]
                tr = trpp.tile([128, 512], DT.bfloat16, name="tr")
                for c, (i0, ni) in enumerate(CHUNKS):
                    nd = ni * NP
                    tin = _ap(dn, i0 * NP, [[ND, 128], [1, nd]])
                    nc.tensor.transpose(
                        _ap(tr, c * 128, [[512, nd], [1, 128]]),
                        tin,
                        ident[:, :],
                    )
                for c, (i0, ni) in enumerate(CHUNKS):
                    nd = ni * NP
                    src = _ap(tr, c * 128, [[512, nd], [1, 128]])
                    dst = _ap(
                        ot,
                        c * 16 * W + hp * W + 32 * wb + wp,
                        [[4 * 16 * W, nd], [2 * W, 8], [2, 16]],
                    )
                    if wb % 2 == 0:
                        nc.vector.tensor_copy(dst, src)
                    else:
                        nc.scalar.activation(dst, src, AF.Copy)

            def stage_store(s):
                k = state[s]["k"]
                ot = out_t[k]
                nc.sync.dma_start(
                    bass.AP(
                        tensor=out_d,
                        offset=(16 * k) * W,
                        ap=[[HW, 126], [126 * HW, 3], [1, 16 * W]],
                    ),
                    _ap(ot, 0, [[4 * 16 * W, 126], [16 * W, 3], [1, 16 * W]]),
                )
                nc.sync.dma_start(
                    bass.AP(
                        tensor=out_d,
                        offset=378 * HW + (16 * k) * W,
                        ap=[[HW, 63], [1, 16 * W]],
                    ),
                    _ap(ot, 3 * 16 * W, [[4 * 16 * W, 63], [1, 16 * W]]),
                )

            LAG = 4
            for s in range(NS + LAG):
                # gathers first: dispatched a full iteration before their
                # compact consumes them (completion latency ~5-15us)
                if 0 <= s - 1 < NS:
                    stage_gather(s - 1)
                # compacts next: they feed transposes two iterations later,
                # and must sit early in the DVE queue to stay ahead of the PE
                if 0 <= s - 2 < NS:
                    stage_compact(s - 2)
                if s < NS:
                    k, sub = s // 4, s % 4
                    if sub == 0:
                        if k + 3 < NBK:
                            load_img_group(k + 3)
                        if k + 1 < NBK:
                            raw1[k + 1] = load_stg1(k + 1)
                    if sub == 2 and k + 1 < NBK:
                        stg1[k + 1] = build_blk(raw1.pop(k + 1))
                # interleave matmuls(s) with transposes(s-LAG) per wb so
                # transpose work fills PSUM-dependency gaps in the PE stream
                for wb in range(4):
                    if s < NS:
                        stage_a_wb(s, wb)
                    if s - LAG >= 0:
                        stage_tp_wb(s - LAG, wb)
                if s - LAG >= 0:
                    if (s - LAG) % 4 == 3:
                        stage_store(s - LAG)
                        stg1.pop((s - LAG) // 4, None)
                    state.pop(s - LAG)

    if waitsplit:
        _split_excess_waits(nc)
    return nc


_NC_CACHE = None


def _get_nc():
    global _NC_CACHE
    if _NC_CACHE is None:
        _NC_CACHE = _build_nc()
    return _NC_CACHE


def kernel(input1, input2):
    input1 = np.ascontiguousarray(np.asarray(input1, dtype=np.float32))
    input2 = np.ascontiguousarray(np.asarray(input2, dtype=np.float32))
    assert input1.shape == (B, C, H, W) and input2.shape == (B, C, H, W)
    nc = _get_nc()
    in_maps = [{"in1": input1[b], "in2": input2[b]} for b in range(B)]
    res = run_bass_kernel_spmd(nc, in_maps, core_ids=list(range(B)))
    return np.stack(
        [np.asarray(res.results[b]["out"]).astype(np.float32) for b in range(B)],
        axis=0,
    )


# revision 39
# speedup vs baseline: 1.2331x; 1.2331x over previous
"""Trainium2 Bass kernel for nn_Correlation (FlowNet-style cost volume).

Problem: input1/input2 [8, 256, 96, 128] f32 ->
         out [8, 441, 96, 128] f32
  out[b, 21*i+j, h, w] = leaky_relu_0.1( (1/256) * sum_c
        in1[b,c,h,w] * in2pad[b,c, h+2i, w+2j] )       (pad 20 each side)

Strategy (data-parallel over B across 8 cores; per core = 1 sample):
  * in2 kept in SBUF as a zero-padded full-res image [136, 168] (bf16,
    per 128-channel chunk). Parity selection happens in the matmul APs
    (stride-2 free dims) -- no on-chip rearrangement at all.
  * Pixel tile = 128 pixels (8 parity rows x 16 parity cols) per
    (block-row k, h-parity hp, w-parity wp, w-block wb). PE computes
    band[pixel, (r, v)] over the 28x36 dilated window (contract C=256,
    bf16, fp32 PSUM, two 504-col PSUM banks).
  * leaky_relu = max(0.1x, x) fused into the PSUM->SBUF pass (DVE stt /
    scalar ACT Lrelu); 1/256 pre-folded into in1 (power of two, exact).
  * The per-pixel alignment base (he*36+we) is baked into the DRAM
    scratch *write* AP (per-partition stagger is legal on the flat DRAM
    side), so the gather back is a plain 2-dim AP with 1.5KB contiguous
    runs -- 1 gather per (k,hp,wp,wb) instead of per-(he,wb).
  * Gathered rows hold (i, v=36) windows; PE transpose selects (i, j<21)
    via a 2-free-dim stationary AP and emits d-major chunks of 126.
  * Output bf16, converted to f32 on host.
"""

import numpy as np

import concourse.bass as bass
import concourse.mybir as mybir
from concourse.tile import TileContext
from concourse.bass_utils import run_bass_kernel_spmd
from concourse.masks import make_identity

DT = mybir.dt
AF = mybir.ActivationFunctionType
ALU = mybir.AluOpType

# ---- problem geometry ----
B, C, H, W = 8, 256, 96, 128
NP = 21                      # displacements per axis
ND = NP * NP                 # 441
CC = 2                       # C chunks of 128
HW = H * W

IMG_H, IMG_W = H + 40, W + 40      # 136 x 168 padded full-res image
IMG_F = IMG_H * IMG_W              # 22848
NBK = 6                            # block rows of 16 full-res rows
NR, NV = 28, 36                    # window rows / cols (parity space)
BAND = NR * NV                     # 1008
RUN = (NP - 1) * NV + NP           # 741 contiguous gather run (t = i*36+j)
SPITCH = 1032                      # scratch row pitch (>= 1029: no row overlap)
SBASE = 7 * NV + 15                # 267 stagger headroom
SROWS = 129                        # scratch rows (>= (SBASE+127*SPITCH+BAND)/SPITCH)
STG1_F = CC * 16 * W               # 4096
# transpose d-chunks (i0, ni): nd = 21*ni
CHUNKS = [(0, 6), (6, 6), (12, 6), (18, 3)]

_MAX_WAITS = 1


def _split_excess_waits(nc):
    """This walrus build accepts only ONE sync-wait per instruction; Tile
    emits multi-waits. Hoist excess waits onto same-engine NOPs inserted
    right before the over-subscribed instruction."""
    nid = 0
    for f in nc.m.functions:
        for blk in f.blocks:
            insts = list(blk.instructions)
            out = []
            changed = False
            for inst in insts:
                si = inst.sync_info
                if si is not None and si.on_wait and len(si.on_wait) > _MAX_WAITS:
                    waits = list(si.on_wait)
                    extra, keep = waits[:-_MAX_WAITS], waits[-_MAX_WAITS:]
                    for k in range(0, len(extra), _MAX_WAITS):
                        nop = mybir.InstNoOp(name=f"I-waitsplit-{nid}", ins=[], outs=[])
                        nid += 1
                        nop.engine = inst.engine
                        nop.sync_info = mybir.SyncInfo(
                            on_wait=extra[k : k + _MAX_WAITS], on_update=[]
                        )
                        out.append(nop)
                        changed = True
                    si.on_wait = keep
                    inst.sync_info = si
                out.append(inst)
            if changed:
                blk.instructions = out
    return nc


def _ap(t, off_extra, dims):
    return bass.AP(tensor=t.tensor, offset=t.offset + off_extra, ap=dims)


def _build_nc(waitsplit=True, use_lrelu=False):
    nc = bass.Bass()
    in1_d = nc.dram_tensor("in1", [C, H, W], DT.float32, kind="ExternalInput")
    in2_d = nc.dram_tensor("in2", [C, H, W], DT.float32, kind="ExternalInput")
    out_d = nc.dram_tensor("out", [ND, H, W], DT.bfloat16, kind="ExternalOutput")

    with TileContext(nc) as tc:
        with (
            tc.tile_pool(name="constp", bufs=1) as constp,
            tc.tile_pool(name="stg1p", bufs=2) as stg1p,
            tc.tile_pool(name="bandp", bufs=4) as bandp,
            tc.tile_pool(name="aligp", bufs=4) as aligp,
            tc.tile_pool(name="outp", bufs=2) as outp,
            tc.tile_pool(name="psp", bufs=3, space="PSUM") as psp,
            tc.tile_pool(name="trpp", bufs=2, space="PSUM") as trpp,
            tc.tile_pool(name="dramp", bufs=10, space="DRAM") as dramp,
        ):
            ident = constp.tile([128, 128], DT.bfloat16)
            make_identity(nc, ident)

            # persistent padded in2 images (one per 128-channel chunk)
            img = [constp.tile([128, IMG_F], DT.bfloat16, name=f"img{cc}") for cc in range(CC)]
            def memset_borders():
                # zero borders: top/bottom row bands, left/right col bands
                # (Pool engine: keeps DVE free for the prologue blk copies)
                for cc in range(CC):
                    t = img[cc]
                    nc.gpsimd.memset(_ap(t, 0, [[IMG_F, 128], [1, 20 * IMG_W]]), 0.0)
                    nc.gpsimd.memset(
                        _ap(t, (IMG_H - 20) * IMG_W, [[IMG_F, 128], [1, 20 * IMG_W]]),
                        0.0,
                    )
                    nc.gpsimd.memset(
                        _ap(t, 20 * IMG_W, [[IMG_F, 128], [IMG_W, 96], [1, 20]]), 0.0
                    )
                    nc.gpsimd.memset(
                        _ap(
                            t,
                            20 * IMG_W + 20 + W,
                            [[IMG_F, 128], [IMG_W, 96], [1, 20]],
                        ),
                        0.0,
                    )

            def load_img_group(g):
                """in2 full-res rows [16g, 16g+16) -> img rows [16g+20, ...)."""
                for cc in range(CC):
                    nc.gpsimd.dma_start(
                        _ap(
                            img[cc],
                            (16 * g + 20) * IMG_W + 20,
                            [[IMG_F, 128], [IMG_W, 16], [1, W]],
                        ),
                        in2_d[cc * 128 : (cc + 1) * 128, 16 * g : 16 * g + 16, :],
                    )

            def load_stg1(k):
                """in1 rows [16k, 16k+16) raw row-major staging."""
                t = stg1p.tile([128, STG1_F], DT.bfloat16, name="stg1")
                for cc in range(CC):
                    nc.gpsimd.dma_start(
                        _ap(t, cc * 16 * W, [[STG1_F, 128], [1, 16 * W]]),
                        in1_d[cc * 128 : (cc + 1) * 128, 16 * k : 16 * k + 16, :],
                    )
                return t

            def build_blk(t):
                """Block stg1 into [c, (cc,hp,wp,wb)*128 pixels] weight tiles,
                prescaled by 1/C (folded into the copy)."""
                blk = stg1p.tile([128, STG1_F], DT.bfloat16, name="in1blk")
                n = 0
                for cc in range(CC):
                    for hp in range(2):
                        for wp in range(2):
                            for wb in range(4):
                                src = _ap(
                                    t,
                                    cc * 16 * W + hp * W + 32 * wb + wp,
                                    [[STG1_F, 128], [2 * W, 8], [2, 16]],
                                )
                                dst = _ap(
                                    blk,
                                    (((cc * 2 + hp) * 2 + wp) * 4 + wb) * 128,
                                    [[STG1_F, 128], [1, 128]],
                                )
                                if n % 2 == 0:
                                    nc.vector.tensor_scalar_mul(dst, src, 1.0 / C)
                                else:
                                    nc.scalar.activation(
                                        dst, src, AF.Copy, scale=1.0 / C
                                    )
                                n += 1
                return blk

            # prologue: in1 slab first (its blk copies gate the first matmuls),
            # then img groups; border memsets ride the Pool queue after the
            # load dispatches
            raw1 = {0: load_stg1(0)}
            for g in range(4):
                load_img_group(g)
            memset_borders()
            stg1 = {0: build_blk(raw1[0])}

            NS = NBK * 4  # 24 supertiles, s = k*4 + hp*2 + wp
            state = {}  # s -> dict with scratch tiles / alig / dense / meta
            out_t = {}

            def stage_a_wb(s, wb):
                """4 matmuls + band copy + scratch write for one wb tile."""
                k, hp, wp = s // 4, (s // 2) % 2, s % 2
                st = state.setdefault(s, {"k": k, "hp": hp, "wp": wp, "scr": [], "dn": []})
                ps = psp.tile([128, 1024], DT.float32, name="ps")
                for bank in range(2):
                    for cc in range(CC):
                        lhsT = _ap(
                            stg1[k],
                            (((cc * 2 + hp) * 2 + wp) * 4 + wb) * 128,
                            [[STG1_F, 128], [1, 128]],
                        )
                        rhs = _ap(
                            img[cc],
                            (16 * k + hp + 2 * 14 * bank) * IMG_W + 32 * wb + wp,
                            [[IMG_F, 128], [2 * IMG_W, 14], [2, NV]],
                        )
                        nc.tensor.matmul(
                            _ap(ps, 512 * bank, [[1024, 128], [1, 504]]),
                            lhsT,
                            rhs,
                            start=(cc == 0),
                            stop=(cc == CC - 1),
                        )
                band = bandp.tile([128, BAND], DT.bfloat16, name="band")
                src = _ap(ps, 0, [[1024, 128], [512, 2], [1, 504]])
                dst = _ap(band, 0, [[BAND, 128], [504, 2], [1, 504]])
                if wb % 2 == 0:
                    nc.vector.tensor_copy(dst, src)
                else:
                    nc.scalar.activation(dst, src, AF.Copy)
                scr = dramp.tile([SROWS, SPITCH], DT.bfloat16, name="scr")
                wdst = _ap(
                    scr,
                    SBASE,
                    [[16 * SPITCH - NV, 8], [SPITCH - 1, 16], [1, BAND]],
                )
                # the write->gather critical chain lives on the sync ring:
                # ring FIFO order makes each gather's write-dependency local
                nc.sync.dma_start(wdst, band[:, :])
                st["scr"].append(scr)

            def stage_gather(s):
                st = state[s]
                alig = aligp.tile([128, 4 * RUN], DT.bfloat16, name="alig")
                for wb in range(4):
                    gsrc = _ap(st["scr"][wb], SBASE, [[SPITCH, 128], [1, RUN]])
                    gdst = _ap(alig, wb * RUN, [[4 * RUN, 128], [1, RUN]])
                    nc.sync.dma_start(gdst, gsrc)
                st["alig"] = alig

            def stage_compact(s):
                """(i, v<36) -> dense (i, j<21) with fused leaky_relu; the
                transpose stationary APs must be single-free-dim (walrus)."""
                st = state[s]
                alig = st["alig"]
                for wb in range(4):
                    dn = aligp.tile([128, ND], DT.bfloat16, name="dense", bufs=16)
                    csrc = _ap(alig, wb * RUN, [[4 * RUN, 128], [NV, NP], [1, NP]])
                    cdst = _ap(dn, 0, [[ND, 128], [1, ND]])
                    if use_lrelu and wb % 2:
                        nc.scalar.activation(cdst, csrc, AF.Lrelu, alpha=0.1)
                    else:
                        nc.vector.scalar_tensor_tensor(
                            cdst, csrc, 0.1, csrc, ALU.mult, ALU.max
                        )
                    st["dn"].append(dn)

            def stage_tp_wb(s, wb):
                """4 transposes + 4 parity-scatter copies for one wb tile."""
                st = state[s]
                k, hp, wp = st["k"], st["hp"], st["wp"]
                if s % 4 == 0 and wb == 0:
                    out_t[k] = outp.tile([128, 4 * 16 * W], DT.bfloat16, name="outt")
                ot = out_t[k]
                dn = st["dn"][wb]
                tr = trpp.tile([128, 512], DT.bfloat16, name="tr")
                for c, (i0, ni) in enumerate(CHUNKS):
                    nd = ni * NP
                    tin = _ap(dn, i0 * NP, [[ND, 128], [1, nd]])
                    nc.tensor.transpose(
                        _ap(tr, c * 128, [[512, nd], [1, 128]]),
                        tin,
                        ident[:, :],
                    )
                for c, (i0, ni) in enumerate(CHUNKS):
                    nd = ni * NP
                    src = _ap(tr, c * 128, [[512, nd], [1, 128]])
                    dst = _ap(
                        ot,
                        c * 16 * W + hp * W + 32 * wb + wp,
                        [[4 * 16 * W, nd], [2 * W, 8], [2, 16]],
                    )
                    if wb % 2 == 0:
                        nc.vector.tensor_copy(dst, src)
                    else:
                        nc.scalar.activation(dst, src, AF.Copy)

            def stage_store(s):
                k = state[s]["k"]
                ot = out_t[k]
                nc.scalar.dma_start(
                    bass.AP(
                        tensor=out_d,
                        offset=(16 * k) * W,
                        ap=[[HW, 126], [126 * HW, 3], [1, 16 * W]],
                    ),
                    _ap(ot, 0, [[4 * 16 * W, 126], [16 * W, 3], [1, 16 * W]]),
                )
                nc.scalar.dma_start(
                    bass.AP(
                        tensor=out_d,
                        offset=378 * HW + (16 * k) * W,
                        ap=[[HW, 63], [1, 16 * W]],
                    ),
                    _ap(ot, 3 * 16 * W, [[4 * 16 * W, 63], [1, 16 * W]]),
                )

            LAG = 5
            for s in range(NS + LAG):
                # gathers first: dispatched two full iterations before their
                # compact consumes them (completion latency ~5-15us)
                if 0 <= s - 1 < NS:
                    stage_gather(s - 1)
                # compacts next: they feed transposes two iterations later,
                # and must sit early in the DVE queue to stay ahead of the PE
                if 0 <= s - 3 < NS:
                    stage_compact(s - 3)
                if s < NS:
                    k, sub = s // 4, s % 4
                    if sub == 0:
                        if k + 3 < NBK:
                            load_img_group(k + 3)
                        if k + 1 < NBK:
                            raw1[k + 1] = load_stg1(k + 1)
                    if sub == 2 and k + 1 < NBK:
                        stg1[k + 1] = build_blk(raw1.pop(k + 1))
                # interleave matmuls(s) with transposes(s-LAG) per wb so
                # transpose work fills PSUM-dependency gaps in the PE stream
                for wb in range(4):
                    if s < NS:
                        stage_a_wb(s, wb)
                    if s - LAG >= 0:
                        stage_tp_wb(s - LAG, wb)
                if s - LAG >= 0:
                    if (s - LAG) % 4 == 3:
                        stage_store(s - LAG)
                        stg1.pop((s - LAG) // 4, None)
                    state.pop(s - LAG)

    if waitsplit:
        _split_excess_waits(nc)
    return nc


_NC_CACHE = None


def _get_nc():
    global _NC_CACHE
    if _NC_CACHE is None:
        _NC_CACHE = _build_nc()
    return _NC_CACHE


def kernel(input1, input2):
    input1 = np.ascontiguousarray(np.asarray(input1, dtype=np.float32))
    input2 = np.ascontiguousarray(np.asarray(input2, dtype=np.float32))
    assert input1.shape == (B, C, H, W) and input2.shape == (B, C, H, W)
    nc = _get_nc()
    in_maps = [{"in1": input1[b], "in2": input2[b]} for b in range(B)]
    res = run_bass_kernel_spmd(nc, in_maps, core_ids=list(range(B)))
    return np.stack(
        [np.asarray(res.results[b]["out"]).astype(np.float32) for b in range(B)],
        axis=0,
    )
